# revision 3
# baseline (speedup 1.0000x reference)
"""Trainium2 kernel for nn_ClusterManager (vq_codebook).

Strategy
--------
The only heavy compute in the module is the per-batch feature Gram matrix
G_b = ff_b @ ff_b.T with ff_b = features[b].reshape(256, 16384) (fp32):
~17 GFLOP total. Everything else (FPS over 256x256 distances, capacity
assignment over 256 channels) is a few hundred KFLOPs of inherently
sequential argmax/scan logic, done on host in fp64.

Data-parallel over batch: core b computes batch b's Gram matrix.

Precision: FPS argmax decision margins on this problem are as small as
~0.18 in squared-distance units, so bf16/fp16 single-pass (err ~ 0.1) and
fp32r (err ~ 3) would flip decisions, while true fp32 matmul is 4x slower
on the PE. We use an fp16 hi/lo two-pass scheme:
    x = hi + lo  (hi = fp16(x), lo = fp16(x - hi), exact to ~2^-21 rel)
    G = hi@hi.T + S + S.T,  S = hi@lo.T   (lo@lo.T term ~ 3e-4, dropped)
Device emits [hi@hi.T | hi@lo.T] in one fp16 N=512 matmul per (k-tile,
m-tile); max |d2| error ~ 1e-4 -- 1000x below the decision margin.
Row norms are computed on host in fp64, so G's diagonal is not used.

Per-core pipeline (128 k-tiles of 128 contraction dims, grouped by 4):
  DMA natural [128c x 512d] fp32 chunks (2 channel halves)
  -> PE transpose per 128x128 block into PSUM (fp32, exact)
  -> ACT: hi = fp16(psum), DVE: lo = fp16(psum - hi) into SBUF [hi|lo]
  -> PE: matmul(psum_g[m], lhsT=hiT[m], rhs=[hiT|loT], N=512) accumulate.
"""

import numpy as np

# ---------------------------------------------------------------- constants
B = 8
C = 256
DF = 16384  # 64 * 256 flattened feature dim
P = 128
KT = DF // P          # 128 k-tiles
GRP = 4               # k-tiles per group
NG = KT // GRP        # 32 groups
LAG = 2               # groups the matmuls trail the transpose/cast stage

NUM_CLUSTERS = 16
UPDATE_RATE = 0.2
_BASE = C // NUM_CLUSTERS
_REM = C % NUM_CLUSTERS
CLUSTER_SIZES = np.array(
    [_BASE + 1] * _REM + [_BASE] * (NUM_CLUSTERS - _REM), dtype=np.int64
)

_CACHED = {}


# ---------------------------------------------------------------- device part
def _build_program():
    import concourse.bass as bass
    import concourse.tile as tile
    from concourse import bacc, mybir
    from concourse.masks import make_identity

    f32 = mybir.dt.float32
    f16 = mybir.dt.float16

    nc = bacc.Bacc("TRN2", target_bir_lowering=False, debug=False, num_devices=B)
    x = nc.dram_tensor("x", [C, DF], f32, kind="ExternalInput").ap()
    g = nc.dram_tensor("g", [C, 2 * C], f32, kind="ExternalOutput").ap()

    with tile.TileContext(nc) as tc:
        with (
            tc.tile_pool(name="const", bufs=1) as const_pool,
            tc.tile_pool(name="nat", bufs=3) as nat_pool,
            tc.tile_pool(name="pt", bufs=2, space="PSUM") as pt_pool,
            tc.tile_pool(name="xt", bufs=LAG + 2) as xt_pool,
            tc.tile_pool(name="gacc", bufs=1, space="PSUM") as gacc_pool,
            tc.tile_pool(name="gout", bufs=1) as gout_pool,
        ):
            ident = const_pool.tile([P, P], f32, tag="ident")
            make_identity(nc, ident[:])

            g_ps = [
                gacc_pool.tile([P, 4 * P], f32, tag=f"g{m}", name=f"g_ps{m}")
                for m in range(2)
            ]

            def stage(gi):
                """DMA + transpose + hi/lo cast for group gi; returns xT tile."""
                xt = xt_pool.tile([P, GRP, 4 * P], f16, tag="xt")
                for h in range(2):
                    nat = nat_pool.tile([P, GRP * P], f32, tag=f"nat{h}")
                    nc.sync.dma_start(
                        nat[:],
                        x[h * P : (h + 1) * P, gi * GRP * P : (gi + 1) * GRP * P],
                    )
                    pt = pt_pool.tile([P, GRP, P], f32, tag=f"pt{h}")
                    for kt in range(GRP):
                        nc.tensor.transpose(
                            pt[:, kt, :],
                            nat[:, kt * P : (kt + 1) * P],
                            ident[:],
                        )
                    # hi = fp16(xT) ; lo = fp16(xT - hi)
                    hi = xt[:, :, h * P : (h + 1) * P]
                    lo = xt[:, :, 2 * P + h * P : 2 * P + (h + 1) * P]
                    nc.scalar.copy(hi, pt[:])
                    nc.vector.tensor_sub(lo, pt[:], hi)
                return xt

            def matmuls(gi, xt):
                for kt in range(GRP):
                    k_idx = gi * GRP + kt
                    for m in range(2):
                        nc.tensor.matmul(
                            g_ps[m][:],
                            lhsT=xt[:, kt, m * P : (m + 1) * P],
                            rhs=xt[:, kt, :],
                            start=(k_idx == 0),
                            stop=(k_idx == KT - 1),
                            skip_group_check=True,
                        )

            pending = []
            for gi in range(NG + LAG):
                if gi < NG:
                    pending.append((gi, stage(gi)))
                if gi >= LAG:
                    matmuls(*pending.pop(0))

            for m in range(2):
                g_sb = gout_pool.tile([P, 4 * P], f32, tag=f"gsb{m}")
                nc.vector.tensor_copy(g_sb[:], g_ps[m][:])
                nc.sync.dma_start(g[m * P : (m + 1) * P, :], g_sb[:])

    nc.compile()
    return nc


def _run_device(ff, trace=False, trace_cores=None):
    """ff: [B, C, DF] fp32 -> (Ghh [B,C,C], S [B,C,C], BassKernelResults)."""
    from concourse.bass_utils import run_bass_kernel_spmd

    if "nc" not in _CACHED:
        _CACHED["nc"] = _build_program()
    nc = _CACHED["nc"]

    in_maps = [{"x": np.ascontiguousarray(ff[b])} for b in range(B)]
    res = run_bass_kernel_spmd(
        nc, in_maps, core_ids=list(range(B)), trace=trace, trace_cores=trace_cores
    )
    gs = np.stack([res.results[b]["g"] for b in range(B)])  # [B, C, 2C]
    return gs[:, :, :C], gs[:, :, C:], res


# ---------------------------------------------------------------- host part
def _cdist(a, b):
    d2 = (
        np.sum(a * a, -1)[..., :, None]
        + np.sum(b * b, -1)[..., None, :]
        - 2.0 * (a @ np.swapaxes(b, -1, -2))
    )
    return np.sqrt(np.clip(d2, 0.0, None))


def _fps_from_D(D, k):
    start = int(np.argmax(D.sum(1)))
    sel = [start]
    min_d = D[start].copy()
    for _ in range(k - 1):
        far = int(np.argmax(min_d))
        sel.append(far)
        min_d = np.minimum(min_d, D[far])
    return np.array(sel)


def _capacity_assign(D, sizes):
    order = np.argsort(D, axis=1, kind="stable")  # [C, K]
    counts = np.zeros(sizes.shape[0], np.int64)
    out = np.empty(D.shape[0], np.int32)
    for ci in range(D.shape[0]):
        row = order[ci]
        chosen = row[int(np.argmax(counts[row] < sizes[row]))]
        counts[chosen] += 1
        out[ci] = chosen
    return out


def _finish(d2_batches, pos_emb_batch):
    pos_emb = pos_emb_batch.astype(np.float64)
    K = NUM_CLUSTERS
    pos = pos_emb[0]
    centers = pos[_fps_from_D(_cdist(pos, pos), K)]
    sels = []
    for bi in range(B):
        d2 = d2_batches[bi].copy()
        np.fill_diagonal(d2, 0.0)
        sels.append(_fps_from_D(np.sqrt(np.clip(d2, 0.0, None)), K))
    sel = np.stack(sels)
    center_coords = pos_emb[np.arange(B)[:, None], sel]
    temp_assign = np.argmin(_cdist(pos_emb, center_coords), -1)
    flat_a = temp_assign.reshape(-1)
    flat_p = pos_emb.reshape(-1, 3)
    sums = np.zeros((K, 3))
    cnts = np.zeros(K)
    np.add.at(sums, flat_a, flat_p)
    np.add.at(cnts, flat_a, 1.0)
    avg = np.where(cnts[:, None] > 0, sums / np.maximum(cnts, 1.0)[:, None], 0.0)
    matching = np.argmin(_cdist(centers, avg), axis=1)
    centers = (1.0 - UPDATE_RATE) * centers + UPDATE_RATE * avg[matching]
    return _capacity_assign(_cdist(pos, centers), CLUSTER_SIZES)


def kernel(features, pos_emb_batch):
    ff = np.asarray(features, dtype=np.float32).reshape(B, C, DF)
    Ghh, S, _ = _run_device(ff)
    ff64 = ff.astype(np.float64)
    n = np.einsum("bcd,bcd->bc", ff64, ff64)
    G = Ghh.astype(np.float64) + S.astype(np.float64) + np.swapaxes(S, 1, 2)
    d2 = n[:, :, None] + n[:, None, :] - 2.0 * G
    return _finish(d2, np.asarray(pos_emb_batch)).astype(np.int32)


# revision 4
# speedup vs baseline: 1.1060x; 1.1060x over previous
"""Trainium2 kernel for nn_ClusterManager (vq_codebook).

Strategy
--------
The only heavy compute in the module is the per-batch feature Gram matrix
G_b = ff_b @ ff_b.T with ff_b = features[b].reshape(256, 16384) (fp32):
~17 GFLOP total. Everything else (FPS over 256x256 distances, capacity
assignment over 256 channels) is a few hundred KFLOPs of inherently
sequential argmax/scan logic, done on host in fp64.

Data-parallel over batch: core b computes batch b's Gram matrix.

Precision: FPS argmax decision margins on this problem are as small as
~0.18 in squared-distance units, so bf16/fp16 single-pass (err ~ 0.1) and
fp32r (err ~ 3) would flip decisions, while true fp32 matmul is 4x slower
on the PE. We use an fp16 hi/lo two-pass scheme:
    x = hi + lo  (hi = fp16(x), lo = fp16(x - hi), exact to ~2^-21 rel)
    G = hi@hi.T + S + S.T,  S = hi@lo.T   (lo@lo.T term ~ 3e-4, dropped)
Max |d2| error ~ 1e-4 -- three orders of magnitude below the decision
margin. Row norms are computed on host in fp64 (G's diagonal unused).

Layout: the host uploads features pre-transposed as [p=128, kt=128, c=256]
(element [p, kt, c] = ff[c, kt*128+p]) so the contraction dim lands on
SBUF partitions with no on-chip transposes and fully contiguous DMA.

Per-core device pipeline (128 k-tiles of 128 contraction dims, by 4):
  DMA [128p x 4kt x 256c] fp32 chunk (4 KB contiguous per partition)
  -> ACT: hi = fp16(x); DVE: lo = fp16(x - hi) into SBUF [hi | lo]
  -> PE: per k-tile,
       mm(out=g0[:, 0:512],  lhsT=hi[:, 0:128], rhs=[hi|lo],  N=512)
       mm(out=g1[:, 128:512], lhsT=hi[:, 128:256], rhs=[hi|lo][128:], N=384)
     accumulating in PSUM over all 128 k-tiles.  The second matmul skips
     the lower-left hi@hi block, which the host restores by symmetry.
"""

import numpy as np

# ---------------------------------------------------------------- constants
B = 8
C = 256
DF = 16384  # 64 * 256 flattened feature dim
P = 128
KT = DF // P          # 128 k-tiles
GRP = 4               # k-tiles per group
NG = KT // GRP        # 32 groups
LAG = 2               # groups the matmuls trail the DMA/cast stage

NUM_CLUSTERS = 16
UPDATE_RATE = 0.2
_BASE = C // NUM_CLUSTERS
_REM = C % NUM_CLUSTERS
CLUSTER_SIZES = np.array(
    [_BASE + 1] * _REM + [_BASE] * (NUM_CLUSTERS - _REM), dtype=np.int64
)

_CACHED = {}


# ---------------------------------------------------------------- device part
def _build_program():
    import concourse.tile as tile
    from concourse import bacc, mybir

    f32 = mybir.dt.float32
    f16 = mybir.dt.float16

    nc = bacc.Bacc("TRN2", target_bir_lowering=False, debug=False, num_devices=B)
    x = nc.dram_tensor("x", [P, KT, C], f32, kind="ExternalInput").ap()
    g = nc.dram_tensor("g", [C, 2 * C], f32, kind="ExternalOutput").ap()

    with tile.TileContext(nc) as tc:
        with (
            tc.tile_pool(name="nat", bufs=3) as nat_pool,
            tc.tile_pool(name="xt", bufs=LAG + 2) as xt_pool,
            tc.tile_pool(name="gacc", bufs=1, space="PSUM") as gacc_pool,
            tc.tile_pool(name="gout", bufs=1) as gout_pool,
        ):
            g_ps = [
                gacc_pool.tile([P, 4 * P], f32, tag=f"g{m}", name=f"g_ps{m}")
                for m in range(2)
            ]

            def stage(gi):
                """DMA + hi/lo cast for group gi; returns fp16 [hi|lo] tile."""
                nat = nat_pool.tile([P, GRP, C], f32, tag="nat")
                nc.sync.dma_start(nat[:], x[:, gi * GRP : (gi + 1) * GRP, :])
                xt = xt_pool.tile([P, GRP, 2 * C], f16, tag="xt")
                hi = xt[:, :, :C]
                lo = xt[:, :, C:]
                nc.scalar.copy(hi, nat[:])          # hi = fp16(x)
                nc.vector.tensor_sub(lo, nat[:], hi)  # lo = fp16(x - hi)
                return xt

            def matmuls(gi, xt):
                for kt in range(GRP):
                    k_idx = gi * GRP + kt
                    start = k_idx == 0
                    stop = k_idx == KT - 1
                    nc.tensor.matmul(
                        g_ps[0][:],
                        lhsT=xt[:, kt, 0:P],
                        rhs=xt[:, kt, :],
                        start=start,
                        stop=stop,
                        skip_group_check=True,
                    )
                    nc.tensor.matmul(
                        g_ps[1][:, P:],
                        lhsT=xt[:, kt, P : 2 * P],
                        rhs=xt[:, kt, P:],
                        start=start,
                        stop=stop,
                        skip_group_check=True,
                    )

            pending = []
            for gi in range(NG + LAG):
                if gi < NG:
                    pending.append((gi, stage(gi)))
                if gi >= LAG:
                    matmuls(*pending.pop(0))

            g_sb0 = gout_pool.tile([P, 4 * P], f32, tag="gsb0")
            nc.scalar.copy(g_sb0[:], g_ps[0][:])
            nc.sync.dma_start(g[:P, :], g_sb0[:])
            g_sb1 = gout_pool.tile([P, 3 * P], f32, tag="gsb1")
            nc.vector.tensor_copy(g_sb1[:], g_ps[1][:, P:])
            nc.sync.dma_start(g[P:, P:], g_sb1[:])

    nc.compile()
    return nc


def _device_layout(ff_b):
    """[C, DF] fp32 -> [P, KT, C] contiguous (d = kt*P + p on partitions)."""
    return np.ascontiguousarray(ff_b.reshape(C, KT, P).transpose(2, 1, 0))


def _run_device(ff, trace=False, trace_cores=None):
    """ff: [B, C, DF] fp32 -> (Ghh [B,C,C], S [B,C,C], BassKernelResults).

    Ghh's lower-left 128x128 block is not computed on device; it is
    restored from the upper-right block by symmetry here.
    """
    from concourse.bass_utils import run_bass_kernel_spmd

    if "nc" not in _CACHED:
        _CACHED["nc"] = _build_program()
    nc = _CACHED["nc"]

    in_maps = [{"x": _device_layout(ff[b])} for b in range(B)]
    res = run_bass_kernel_spmd(
        nc, in_maps, core_ids=list(range(B)), trace=trace, trace_cores=trace_cores
    )
    gs = np.stack([res.results[b]["g"] for b in range(B)])  # [B, C, 2C]
    Ghh = gs[:, :, :C].copy()
    Ghh[:, P:, :P] = np.swapaxes(Ghh[:, :P, P:], 1, 2)
    return Ghh, gs[:, :, C:], res


# ---------------------------------------------------------------- host part
def _cdist(a, b):
    d2 = (
        np.sum(a * a, -1)[..., :, None]
        + np.sum(b * b, -1)[..., None, :]
        - 2.0 * (a @ np.swapaxes(b, -1, -2))
    )
    return np.sqrt(np.clip(d2, 0.0, None))


def _fps_from_D(D, k):
    start = int(np.argmax(D.sum(1)))
    sel = [start]
    min_d = D[start].copy()
    for _ in range(k - 1):
        far = int(np.argmax(min_d))
        sel.append(far)
        min_d = np.minimum(min_d, D[far])
    return np.array(sel)


def _capacity_assign(D, sizes):
    order = np.argsort(D, axis=1, kind="stable")  # [C, K]
    counts = np.zeros(sizes.shape[0], np.int64)
    out = np.empty(D.shape[0], np.int32)
    for ci in range(D.shape[0]):
        row = order[ci]
        chosen = row[int(np.argmax(counts[row] < sizes[row]))]
        counts[chosen] += 1
        out[ci] = chosen
    return out


def _finish(d2_batches, pos_emb_batch):
    pos_emb = pos_emb_batch.astype(np.float64)
    K = NUM_CLUSTERS
    pos = pos_emb[0]
    centers = pos[_fps_from_D(_cdist(pos, pos), K)]
    sels = []
    for bi in range(B):
        d2 = d2_batches[bi].copy()
        np.fill_diagonal(d2, 0.0)
        sels.append(_fps_from_D(np.sqrt(np.clip(d2, 0.0, None)), K))
    sel = np.stack(sels)
    center_coords = pos_emb[np.arange(B)[:, None], sel]
    temp_assign = np.argmin(_cdist(pos_emb, center_coords), -1)
    flat_a = temp_assign.reshape(-1)
    flat_p = pos_emb.reshape(-1, 3)
    sums = np.zeros((K, 3))
    cnts = np.zeros(K)
    np.add.at(sums, flat_a, flat_p)
    np.add.at(cnts, flat_a, 1.0)
    avg = np.where(cnts[:, None] > 0, sums / np.maximum(cnts, 1.0)[:, None], 0.0)
    matching = np.argmin(_cdist(centers, avg), axis=1)
    centers = (1.0 - UPDATE_RATE) * centers + UPDATE_RATE * avg[matching]
    return _capacity_assign(_cdist(pos, centers), CLUSTER_SIZES)


def kernel(features, pos_emb_batch):
    ff = np.asarray(features, dtype=np.float32).reshape(B, C, DF)
    Ghh, S, _ = _run_device(ff)
    ff64 = ff.astype(np.float64)
    n = np.einsum("bcd,bcd->bc", ff64, ff64)
    G = Ghh.astype(np.float64) + S.astype(np.float64) + np.swapaxes(S, 1, 2)
    d2 = n[:, :, None] + n[:, None, :] - 2.0 * G
    return _finish(d2, np.asarray(pos_emb_batch)).astype(np.int32)


# revision 9
# speedup vs baseline: 1.4945x; 1.3513x over previous
"""Trainium2 kernel for nn_ClusterManager (vq_codebook).

Strategy
--------
The only heavy compute in the module is the per-batch feature Gram matrix
G_b = ff_b @ ff_b.T with ff_b = features[b].reshape(256, 16384) (fp32):
~17 GFLOP total. Everything else (FPS over 256x256 distances, capacity
assignment over 256 channels) is a few hundred KFLOPs of inherently
sequential argmax/scan logic, done on host in fp64.

Data-parallel over batch: core b computes batch b's Gram matrix.

Precision: FPS argmax decision margins on this problem are as small as
~0.18 in squared-distance units, so bf16/fp16 single-pass (err ~ 0.1) and
fp32r (err ~ 3) would flip decisions, while true fp32 matmul is 4x slower
on the PE. We use an fp16 hi/lo two-pass scheme:
    x = hi + lo  (hi = fp16(x), lo = fp16(x - hi), exact to ~2^-21 rel)
    G = hi@hi.T + S + S.T,  S = hi@lo.T   (lo@lo.T term ~ 3e-4, dropped)
Max |d2| error ~ 1e-4 -- three orders of magnitude below the decision
margin. Row norms are computed on host in fp64 (G's diagonal unused).

Layout: the host uploads features pre-transposed as [p=128, kt=128, c=256]
(element [p, kt, c] = ff[c, kt*128+p]) so the contraction dim lands on
SBUF partitions with no on-chip transposes and fully contiguous DMA.

Per-core device pipeline (128 k-tiles of 128 contraction dims, by 4):
  DMA [128p x 4kt x 256c] fp32 chunk (4 KB contiguous per partition)
  -> ACT: hi = fp16(x); DVE: lo = fp16(x - hi) into SBUF [hi | lo]
  -> PE: per k-tile,
       mm(out=g0[:, 0:512],  lhsT=hi[:, 0:128], rhs=[hi|lo],  N=512)
       mm(out=g1[:, 128:512], lhsT=hi[:, 128:256], rhs=[hi|lo][128:], N=384)
     accumulating in PSUM over all 128 k-tiles.  The second matmul skips
     the lower-left hi@hi block, which the host restores by symmetry.
"""

import numpy as np

# ---------------------------------------------------------------- constants
B = 8
C = 256
DF = 16384  # 64 * 256 flattened feature dim
P = 128
KT = DF // P          # 128 k-tiles
GRP = 2               # k-tiles per group
NG = KT // GRP        # 64 groups
LAG = 6               # groups the matmuls trail the DMA/cast stage

NUM_CLUSTERS = 16
UPDATE_RATE = 0.2
_BASE = C // NUM_CLUSTERS
_REM = C % NUM_CLUSTERS
CLUSTER_SIZES = np.array(
    [_BASE + 1] * _REM + [_BASE] * (NUM_CLUSTERS - _REM), dtype=np.int64
)

_CACHED = {}


# ---------------------------------------------------------------- device part
def _build_program():
    import concourse.tile as tile
    from concourse import bacc, mybir

    f32 = mybir.dt.float32
    f16 = mybir.dt.float16

    nc = bacc.Bacc(
        "TRN2",
        target_bir_lowering=False,
        debug=False,
        enable_asserts=False,
        num_devices=B,
    )
    x = nc.dram_tensor("x", [P, KT, C], f32, kind="ExternalInput").ap()
    g = nc.dram_tensor("g", [C, 2 * C], f32, kind="ExternalOutput").ap()

    with tile.TileContext(nc) as tc:
        with (
            tc.tile_pool(name="nat", bufs=LAG + 2) as nat_pool,
            tc.tile_pool(name="xt", bufs=LAG + 2) as xt_pool,
            tc.tile_pool(name="gacc", bufs=1, space="PSUM") as gacc_pool,
            tc.tile_pool(name="gout", bufs=1) as gout_pool,
        ):
            g_ps = [
                gacc_pool.tile([P, 4 * P], f32, tag=f"g{m}", name=f"g_ps{m}")
                for m in range(2)
            ]

            def stage(gi):
                """DMA + hi/lo cast for group gi; returns fp16 [hi|lo] tile."""
                nat = nat_pool.tile([P, GRP, C], f32, tag="nat")
                nc.sync.dma_start(nat[:], x[:, gi * GRP : (gi + 1) * GRP, :])
                xt = xt_pool.tile([P, GRP, 2 * C], f16, tag="xt")
                hi = xt[:, :, :C]
                lo = xt[:, :, C:]
                nc.scalar.copy(hi, nat[:])          # hi = fp16(x)
                nc.vector.tensor_sub(lo, nat[:], hi)  # lo = fp16(x - hi)
                return xt

            def matmuls(gi, xt):
                for kt in range(GRP):
                    k_idx = gi * GRP + kt
                    start = k_idx == 0
                    stop = k_idx == KT - 1
                    nc.tensor.matmul(
                        g_ps[0][:],
                        lhsT=xt[:, kt, 0:P],
                        rhs=xt[:, kt, :],
                        start=start,
                        stop=stop,
                        skip_group_check=True,
                    )
                    nc.tensor.matmul(
                        g_ps[1][:, P:],
                        lhsT=xt[:, kt, P : 2 * P],
                        rhs=xt[:, kt, P:],
                        start=start,
                        stop=stop,
                        skip_group_check=True,
                    )

            pending = []
            for gi in range(NG + LAG):
                if gi < NG:
                    pending.append((gi, stage(gi)))
                if gi >= LAG:
                    matmuls(*pending.pop(0))

            g_sb0 = gout_pool.tile([P, 4 * P], f32, tag="gsb0")
            nc.scalar.copy(g_sb0[:], g_ps[0][:])
            nc.sync.dma_start(g[:P, :], g_sb0[:])
            g_sb1 = gout_pool.tile([P, 3 * P], f32, tag="gsb1")
            nc.vector.tensor_copy(g_sb1[:], g_ps[1][:, P:])
            nc.sync.dma_start(g[P:, P:], g_sb1[:])

    nc.compile()
    return nc


def _device_layout(ff_b):
    """[C, DF] fp32 -> [P, KT, C] contiguous (d = kt*P + p on partitions)."""
    return np.ascontiguousarray(ff_b.reshape(C, KT, P).transpose(2, 1, 0))


def _run_device(ff, trace=False, trace_cores=None):
    """ff: [B, C, DF] fp32 -> (Ghh [B,C,C], S [B,C,C], BassKernelResults).

    Ghh's lower-left 128x128 block is not computed on device; it is
    restored from the upper-right block by symmetry here.
    """
    from concourse.bass_utils import run_bass_kernel_spmd

    if "nc" not in _CACHED:
        _CACHED["nc"] = _build_program()
    nc = _CACHED["nc"]

    in_maps = [{"x": _device_layout(ff[b])} for b in range(B)]
    res = run_bass_kernel_spmd(
        nc, in_maps, core_ids=list(range(B)), trace=trace, trace_cores=trace_cores
    )
    gs = np.stack([res.results[b]["g"] for b in range(B)])  # [B, C, 2C]
    Ghh = gs[:, :, :C].copy()
    Ghh[:, P:, :P] = np.swapaxes(Ghh[:, :P, P:], 1, 2)
    return Ghh, gs[:, :, C:], res


# ---------------------------------------------------------------- host part
def _cdist(a, b):
    d2 = (
        np.sum(a * a, -1)[..., :, None]
        + np.sum(b * b, -1)[..., None, :]
        - 2.0 * (a @ np.swapaxes(b, -1, -2))
    )
    return np.sqrt(np.clip(d2, 0.0, None))


def _fps_from_D(D, k):
    start = int(np.argmax(D.sum(1)))
    sel = [start]
    min_d = D[start].copy()
    for _ in range(k - 1):
        far = int(np.argmax(min_d))
        sel.append(far)
        min_d = np.minimum(min_d, D[far])
    return np.array(sel)


def _capacity_assign(D, sizes):
    order = np.argsort(D, axis=1, kind="stable")  # [C, K]
    counts = np.zeros(sizes.shape[0], np.int64)
    out = np.empty(D.shape[0], np.int32)
    for ci in range(D.shape[0]):
        row = order[ci]
        chosen = row[int(np.argmax(counts[row] < sizes[row]))]
        counts[chosen] += 1
        out[ci] = chosen
    return out


def _finish(d2_batches, pos_emb_batch):
    pos_emb = pos_emb_batch.astype(np.float64)
    K = NUM_CLUSTERS
    pos = pos_emb[0]
    centers = pos[_fps_from_D(_cdist(pos, pos), K)]
    sels = []
    for bi in range(B):
        d2 = d2_batches[bi].copy()
        np.fill_diagonal(d2, 0.0)
        sels.append(_fps_from_D(np.sqrt(np.clip(d2, 0.0, None)), K))
    sel = np.stack(sels)
    center_coords = pos_emb[np.arange(B)[:, None], sel]
    temp_assign = np.argmin(_cdist(pos_emb, center_coords), -1)
    flat_a = temp_assign.reshape(-1)
    flat_p = pos_emb.reshape(-1, 3)
    sums = np.zeros((K, 3))
    cnts = np.zeros(K)
    np.add.at(sums, flat_a, flat_p)
    np.add.at(cnts, flat_a, 1.0)
    avg = np.where(cnts[:, None] > 0, sums / np.maximum(cnts, 1.0)[:, None], 0.0)
    matching = np.argmin(_cdist(centers, avg), axis=1)
    centers = (1.0 - UPDATE_RATE) * centers + UPDATE_RATE * avg[matching]
    return _capacity_assign(_cdist(pos, centers), CLUSTER_SIZES)


def kernel(features, pos_emb_batch):
    ff = np.asarray(features, dtype=np.float32).reshape(B, C, DF)
    Ghh, S, _ = _run_device(ff)
    ff64 = ff.astype(np.float64)
    n = np.einsum("bcd,bcd->bc", ff64, ff64)
    G = Ghh.astype(np.float64) + S.astype(np.float64) + np.swapaxes(S, 1, 2)
    d2 = n[:, :, None] + n[:, None, :] - 2.0 * G
    return _finish(d2, np.asarray(pos_emb_batch)).astype(np.int32)
